# revision 7
# baseline (speedup 1.0000x reference)
"""BFP-quantized linear (nn_BFPLinear) on 8 Trainium2 NeuronCores.

Math (must match reference exactly):
    xq = bfp_quant8_g64(x); wq = bfp_quant8_g64(weight)
    out = xq @ wq.T + 2*bias

Sharding (2 row-groups x 4 col-groups grid, core c = 4r+k):
  - x row-shard r = x[2048r:2048(r+1)] is needed by the 4 cores of row-group r.
    Each core quantizes only its OWN 512 rows of x (rows [512c, 512c+512)),
    then AllGathers bf16 xq within its row group in 8 chunks of 64 rows.
  - weight col-shard k = w[1024k:1024(k+1)] is quantized redundantly by the
    2 cores {k, k+4} that need it (cheaper than a second collective).
  - Output shard per core: [2048, 1024] (rows of row-group r in AG chunk
    order, cols 1024k..1024k+1024). Host undoes the chunk permutation.

Quantization per group of 64 along `in`:
    gmax = max|x|; e = floor(log2(gmax)) via exponent-bit masking;
    step = 2^(e-7), inv = 2^(7-e) (exact bit arithmetic);
    m = clip(round_half_even(x*inv), -128, 127) via pre-clip to
    [-128.49, 127.49] + the fp32 magic-number round (+-1.5*2^23);
    xq = m * step, exact in bf16 (|m| <= 128, step = power of 2).
The bf16 matmul inputs are bit-exact equal to the reference's fp32
quantized values, so the only output error is fp32 summation order.
"""
import sys

sys.path.insert(0, "/opt/trn_rl_repo")

import numpy as np
import ml_dtypes

import concourse.bass as bass
import concourse.tile as tile
from concourse import mybir, bacc
from concourse.bass_utils import run_bass_kernel_spmd

# problem shape (hardcoded; kernel.py must be self-contained)
N = 4096
IN = 4096
OUT = 4096
NCORES = 8
RGRP = 2            # row groups (x sharded 2-way for the matmul)
CGRP = 4            # col groups (weight sharded 4-way)
XOWN = N // NCORES          # 512 rows of x quantized per core
WSH = OUT // CGRP           # 1024 weight rows per core
NLOC = N // RGRP            # 2048 output rows per core
J = 64                      # bfp group size
KT = IN // 128              # 32 k-tiles
NCHUNK = 8                  # allgather chunks per core
CH_OWN = XOWN // NCHUNK     # 64 own rows per chunk
CH_ROWS = CH_OWN * CGRP     # 256 gathered rows per chunk
HF = 2048                   # quantize sub-tile width (along `in`)
OHALF = 512                 # output column half processed per psum group

MASK_EXP = 0x7F800000
MIN_NORM = 0x00800000
STEP_SUB = 7 << 23
INV_C = 0x7F000000
MAGIC = float(np.float32(1.5 * 2.0 ** 23))
CLIP_HI = float(np.float32(127.49))
CLIP_LO = float(np.float32(-128.49))

_CACHE = {}


def _emit_quant_half(nc, sb, src_f32, dst_bf16, tag):
    """Quantize a [128, HF] fp32 tile AP into a [128, HF] bf16 AP."""
    dt = mybir.dt
    P = 128
    g = HF // J
    x3 = src_f32.rearrange("p (g j) -> p g j", j=J)

    gmax = sb.tile([P, g], dt.float32, tag=f"gmax{tag}")
    nc.vector.tensor_reduce(gmax[:], x3, mybir.AxisListType.X,
                            mybir.AluOpType.max, apply_absolute_value=True)
    p2 = sb.tile([P, g], dt.int32, tag=f"p2{tag}")
    nc.vector.tensor_scalar(p2[:], gmax[:].bitcast(dt.int32), MASK_EXP, None,
                            mybir.AluOpType.bitwise_and)
    nc.vector.tensor_scalar(p2[:], p2[:], MIN_NORM, None, mybir.AluOpType.max)
    inv_i = sb.tile([P, g], dt.int32, tag=f"invi{tag}")
    nc.vector.tensor_scalar(inv_i[:], p2[:], -1, INV_C,
                            mybir.AluOpType.mult, mybir.AluOpType.add)
    inv_f = sb.tile([P, g], dt.float32, tag=f"invf{tag}")
    nc.vector.tensor_scalar(inv_f[:], inv_i[:].bitcast(dt.float32), 128.0, None,
                            mybir.AluOpType.mult)
    step_i = sb.tile([P, g], dt.int32, tag=f"stepi{tag}")
    nc.vector.tensor_scalar(step_i[:], p2[:], STEP_SUB, None,
                            mybir.AluOpType.subtract)
    step_bf = sb.tile([P, g], dt.bfloat16, tag=f"stepbf{tag}")
    nc.vector.tensor_copy(step_bf[:], step_i[:].bitcast(dt.float32))

    yt = sb.tile([P, HF], dt.float32, tag=f"y{tag}")
    y3 = yt[:].rearrange("p (g j) -> p g j", j=J)
    inv_b = inv_f[:].unsqueeze(2).broadcast_to([P, g, J])
    nc.gpsimd.tensor_tensor(y3, x3, inv_b, mybir.AluOpType.mult)
    nc.vector.tensor_scalar(yt[:], yt[:], CLIP_HI, CLIP_LO,
                            mybir.AluOpType.min, mybir.AluOpType.max)
    mt = sb.tile([P, HF], dt.bfloat16, tag=f"m{tag}")
    nc.vector.tensor_scalar(mt[:], yt[:], MAGIC, MAGIC,
                            mybir.AluOpType.add, mybir.AluOpType.subtract)
    dst3 = dst_bf16.rearrange("p (g j) -> p g j", j=J)
    step_b = step_bf[:].unsqueeze(2).broadcast_to([P, g, J])
    nc.vector.tensor_tensor(dst3, mt[:].rearrange("p (g j) -> p g j", j=J),
                            step_b, mybir.AluOpType.mult)


def build():
    dt = mybir.dt
    P = 128
    nc = bacc.Bacc("TRN2", target_bir_lowering=False, debug=False,
                   num_devices=NCORES)
    x_d = nc.dram_tensor("x_own", [XOWN, IN], dt.float32,
                         kind="ExternalInput").ap()
    w_d = nc.dram_tensor("w_shard", [WSH, IN], dt.float32,
                         kind="ExternalInput").ap()
    b_d = nc.dram_tensor("bias_rep", [P, WSH], dt.float32,
                         kind="ExternalInput").ap()
    out_d = nc.dram_tensor("out", [NLOC, WSH], dt.float32,
                           kind="ExternalOutput").ap()

    groups = [[0, 1, 2, 3], [4, 5, 6, 7]]

    with tile.TileContext(nc) as tc:
        with tc.tile_pool(name="sb", bufs=1) as sb, \
             tc.tile_pool(name="inp", bufs=3) as inp, \
             tc.tile_pool(name="work", bufs=2) as work, \
             tc.tile_pool(name="otp", bufs=4) as otp, \
             tc.tile_pool(name="dramp", bufs=1, space="DRAM") as dramp, \
             tc.tile_pool(name="psum", bufs=6, space="PSUM") as psump:

            # bias * 2 (reference adds bias twice)
            bias_sb = sb.tile([P, WSH], dt.float32)
            nc.sync.dma_start(bias_sb[:], b_d)
            bias2 = sb.tile([P, WSH], dt.float32)
            nc.vector.tensor_scalar(bias2[:], bias_sb[:], 2.0, None,
                                    mybir.AluOpType.mult)

            # wqT resident: [p, kt, jo, o] ; logical weight row = 128*jo + o
            # for col index kt*128 + p
            wqT = sb.tile([P, KT, WSH // P, P], dt.bfloat16)

            # x quantize staging
            xq_own = dramp.tile([XOWN, IN], dt.bfloat16)
            xq_gath = []
            for j in range(NCHUNK):
                g_t = dramp.tile([CH_ROWS, IN], dt.bfloat16, tag=f"gath{j}",
                                 name=f"gath{j}")
                xq_gath.append(g_t)

            def emit_w_tile(jo):
                for h in range(2):
                    wt = inp.tile([P, HF], dt.float32, tag="in")
                    nc.sync.dma_start(wt[:], w_d[jo * P:(jo + 1) * P,
                                                 h * HF:(h + 1) * HF])
                    wq = work.tile([P, HF], dt.bfloat16, tag="wq")
                    _emit_quant_half(nc, work, wt[:], wq[:], "")
                    # transpose into wqT[:, 16h:16h+16, jo, :]
                    nc.sync.dma_start_transpose(
                        wqT[:, 16 * h:16 * (h + 1), jo, :], wq[:])

            def emit_x_slice(s):
                qs = work.tile([P, IN], dt.bfloat16, tag="q")
                for h in range(2):
                    xt = inp.tile([P, HF], dt.float32, tag="in")
                    nc.sync.dma_start(xt[:], x_d[s * P:(s + 1) * P,
                                                 h * HF:(h + 1) * HF])
                    _emit_quant_half(nc, work, xt[:], qs[:, h * HF:(h + 1) * HF], "")
                nc.scalar.dma_start(xq_own[s * P:(s + 1) * P, :], qs[:])
                for jj in (2 * s, 2 * s + 1):
                    nc.gpsimd.collective_compute(
                        "AllGather", mybir.AluOpType.bypass,
                        replica_groups=groups,
                        ins=[xq_own[jj * CH_OWN:(jj + 1) * CH_OWN, :]],
                        outs=[xq_gath[jj][:]],
                    )

            # interleave: first the W tiles feeding o-half 0 and x slice 0,
            # then the rest; Tile's scheduler overlaps by dataflow.
            emit_x_slice(0)
            for jo in range(4):
                emit_w_tile(jo)
            emit_x_slice(1)
            for jo in range(4, 8):
                emit_w_tile(jo)
            emit_x_slice(2)
            emit_x_slice(3)

            # matmul chunks
            for j in range(NCHUNK):
                xqT = work.tile([P, KT, CH_ROWS], dt.bfloat16, tag="xqT")
                nc.sync.dma_start_transpose(xqT[:], xq_gath[j][:])
                for oh in range(2):
                    for nb in range(CH_ROWS // P):
                        ps = psump.tile([P, OHALF], dt.float32, tag="ps")
                        for kt in range(KT):
                            nc.tensor.matmul(
                                ps[:],
                                xqT[:, kt, nb * P:(nb + 1) * P],
                                wqT[:, kt, 4 * oh:4 * (oh + 1), :],
                                start=(kt == 0), stop=(kt == KT - 1),
                            )
                        ot = otp.tile([P, OHALF], dt.float32, tag="ot")
                        nc.vector.tensor_tensor(
                            ot[:], ps[:], bias2[:, oh * OHALF:(oh + 1) * OHALF],
                            mybir.AluOpType.add)
                        nc.scalar.dma_start(
                            out_d[j * CH_ROWS + nb * P:j * CH_ROWS + (nb + 1) * P,
                                  oh * OHALF:(oh + 1) * OHALF],
                            ot[:])
    nc.compile()
    return nc


def _get_nc():
    if "nc" not in _CACHE:
        _CACHE["nc"] = build()
    return _CACHE["nc"]


def time_kernel(x, weight, bias, chain=9, reps=3):
    """Estimate per-execution device time by chaining `chain` NEFF
    executions inside one jit (serialized via a bias data dependency) and
    differencing against a 1-execution call."""
    import time
    import jax
    from jax.sharding import Mesh, PartitionSpec
    from jax.experimental.shard_map import shard_map
    from concourse import bass2jax, mybir as mb

    bass2jax.install_neuronx_cc_hook()
    nc = _get_nc()

    partition_name = (nc.partition_id_tensor.name
                      if nc.partition_id_tensor else None)
    in_names, out_names, out_avals, zero_outs = [], [], [], []
    for alloc in nc.m.functions[0].allocations:
        if not isinstance(alloc, mb.MemoryLocationSet):
            continue
        name = alloc.memorylocations[0].name
        if alloc.kind == "ExternalInput":
            if name != partition_name:
                in_names.append(name)
        elif alloc.kind == "ExternalOutput":
            out_names.append(name)
            shape = tuple(alloc.tensor_shape)
            dtype = mb.dt.np(alloc.dtype)
            out_avals.append(jax.core.ShapedArray(shape, dtype))
            zero_outs.append(np.zeros(shape, dtype))
    n_params = len(in_names)
    all_names = tuple(in_names + out_names
                      + ([partition_name] if partition_name else []))

    def make_body(k):
        def body(*args):
            ins = list(args[:n_params])
            zeros = list(args[n_params:])
            outs = None
            for i in range(k):
                ins_i = list(ins)
                if outs is not None:
                    # serialize: bias input depends on previous output
                    bi = in_names.index("bias_rep")
                    ins_i[bi] = ins_i[bi] + outs[0][0, 0] * 0.0
                extra = ([bass2jax.partition_id_tensor()]
                         if partition_name else [])
                outs = bass2jax._bass_exec_p.bind(
                    *ins_i, *zeros, *extra,
                    out_avals=tuple(out_avals),
                    in_names=all_names,
                    out_names=tuple(out_names),
                    lowering_input_output_aliases=(),
                    sim_require_finite=True,
                    sim_require_nnan=True,
                    nc=nc,
                )
            return tuple(outs)
        return body

    x = np.ascontiguousarray(np.asarray(x, dtype=np.float32))
    weight = np.ascontiguousarray(np.asarray(weight, dtype=np.float32))
    bias = np.asarray(bias, dtype=np.float32)
    per_core = []
    for c in range(NCORES):
        k = c % CGRP
        m = {
            "x_own": x[XOWN * c:XOWN * (c + 1)],
            "w_shard": weight[WSH * k:WSH * (k + 1)],
            "bias_rep": np.ascontiguousarray(
                np.broadcast_to(bias[WSH * k:WSH * (k + 1)], (128, WSH))),
        }
        per_core.append([m[n] for n in in_names])

    devices = jax.devices()[:NCORES]
    mesh = Mesh(np.asarray(devices), ("core",))
    specs = (PartitionSpec("core"),) * (n_params + len(out_names))
    concat_in = [np.concatenate([per_core[c][i] for c in range(NCORES)], axis=0)
                 for i in range(n_params)]
    concat_zeros = [np.zeros((NCORES * z.shape[0], *z.shape[1:]), z.dtype)
                    for z in zero_outs]

    from jax.sharding import NamedSharding
    fn = jax.jit(shard_map(make_body(1), mesh=mesh, in_specs=specs,
                           out_specs=(PartitionSpec("core"),) * len(out_names),
                           check_rep=False), keep_unused=True)
    sh = NamedSharding(mesh, PartitionSpec("core"))
    dev_in = [jax.device_put(a, sh) for a in concat_in]
    dev_zeros = [jax.device_put(a, sh) for a in concat_zeros]
    out = fn(*dev_in, *dev_zeros)      # compile + warm
    jax.block_until_ready(out)
    times = []
    for _ in range(max(reps, 8)):
        t0 = time.perf_counter()
        out = fn(*dev_in, *dev_zeros)
        jax.block_until_ready(out)
        times.append(time.perf_counter() - t0)
    return min(times), sorted(times)


def kernel(x, weight, bias, _trace=False):
    nc = _get_nc()
    x = np.ascontiguousarray(np.asarray(x, dtype=np.float32))
    weight = np.ascontiguousarray(np.asarray(weight, dtype=np.float32))
    bias = np.asarray(bias, dtype=np.float32)

    in_maps = []
    for c in range(NCORES):
        k = c % CGRP
        in_maps.append({
            "x_own": x[XOWN * c:XOWN * (c + 1)],
            "w_shard": weight[WSH * k:WSH * (k + 1)],
            "bias_rep": np.ascontiguousarray(
                np.broadcast_to(bias[WSH * k:WSH * (k + 1)], (128, WSH))),
        })

    res = run_bass_kernel_spmd(nc, in_maps, core_ids=list(range(NCORES)),
                               trace=_trace)
    out = np.empty((N, OUT), dtype=np.float32)
    for c in range(NCORES):
        r, k = c // CGRP, c % CGRP
        shard = res.results[c]["out"]               # [2048, 1024]
        # shard row = 256*j + 64*g + rr  ->  global row 2048*r + 512*g + 64*j + rr
        blk = shard.reshape(NCHUNK, CGRP, CH_OWN, WSH).transpose(1, 0, 2, 3)
        out[NLOC * r:NLOC * (r + 1), WSH * k:WSH * (k + 1)] = \
            blk.reshape(NLOC, WSH)
    if _trace:
        return out, res
    return out
